# revision 10
# baseline (speedup 1.0000x reference)
"""BinaryDiff kernel for Trainium2 (8 NeuronCores).

Computes out = x @ base + coeff * (x @ (2*mask - 1)) by folding the two
matmuls into one:  out = x @ W,  W = base + coeff*(2*mask - 1).

Sharding (8 cores = 2 row-groups x 4 col-groups):
  - x rows (B*S = 8192) split in 2 -> each core gets xT shard [4096, 4096]
    (x is transposed on host so the contraction dim lands on SBUF partitions)
  - base/mask cols (4096) split in 4 -> per-core shards [4096, 1024]
  - each core computes out shard [4096, 1024]; host concatenates.

On-device per core:
  - W = base + (2c*mask - c) built once into resident SBUF ([128,32,1024]
    fp32r), via ACT affine (int32->f32) + DVE add.
  - x^T streamed in 32 slabs [128,32,128]; 32 m-strips x 2 n-halves x 32
    k-chunks of fp32r matmuls (moving dim 512) accumulate in 8 PSUM banks.
  - ACT copies PSUM->SBUF, gpsimd DMAs results out.

Raw bass with manual semaphores (one wait per standalone wait_ge
instruction -- this walrus build allows only ONE sync wait on any engine
datapath instruction, so all multi-dependency sync is expressed as
chains of wait_ge on the consuming engine).
"""
import contextlib

import numpy as np

import concourse.bass as bass
import concourse.mybir as mybir
from concourse.bass_utils import run_bass_kernel_spmd

f32 = mybir.dt.float32
f32r = mybir.dt.float32r
i32 = mybir.dt.int32
Copy = mybir.ActivationFunctionType.Copy
Identity = mybir.ActivationFunctionType.Identity

P = 128
B, S, D_IN, D_OUT = 4, 2048, 4096, 4096
ROWS = B * S                  # 8192
R_SHARDS, C_SHARDS = 2, 4
M = ROWS // R_SHARDS          # 4096 rows per core
NC = D_OUT // C_SHARDS        # 1024 cols per core
K = D_IN                      # 4096 contraction
KT = K // P                   # 32 k-chunks
MS = M // P                   # 32 m-strips
NH = NC // 512                # 2 n-halves
NT = 512
N_PIECES = KT * NH            # 64 W build pieces (k-major: piece j -> k=j//2, h=j%2)
N_GROUPS = MS * NH            # 64 output groups
SLAB_BUFS = 2
CHUNK_BUFS = 3
OUT_BUFS = 4
PSB = 8                       # psum banks in rotation
XT_LANES = 4                  # slab DMA sem lanes (> SLAB_BUFS)
PIECE_LANES = 8               # W piece DMA sem lanes (> CHUNK_BUFS)
OD_LANES = 8                  # out DMA sem lanes (> OUT_BUFS)


def _build_program():
    nc = bass.Bass()
    xT = nc.declare_dram_parameter("xT", [K, M], f32r, isOutput=False)
    base = nc.declare_dram_parameter("base", [K, NC], f32, isOutput=False)
    mask = nc.declare_dram_parameter("mask", [K, NC], i32, isOutput=False)
    coeff = nc.declare_dram_parameter("coeff", [P, 1], f32, isOutput=False)
    out = nc.declare_dram_parameter("out", [M, NC], f32, isOutput=True)

    xT3 = xT.rearrange("(ko p) m -> p ko m", p=P)
    base3 = base.rearrange("(ko p) n -> p ko n", p=P)
    mask3 = mask.rearrange("(ko p) n -> p ko n", p=P)
    out3 = out.rearrange("(mo p) n -> p mo n", p=P)

    with contextlib.ExitStack() as ctx:
        # NOTE: DMA completions across different HW queues are NOT ordered, so
        # a single cumulative semaphore over many in-flight DMAs is racy
        # ("N completions happened" != "the first N completed"). Engine
        # (PE/ACT/DVE) completions retire in order, so cumulative engine sems
        # are sound. Hence: dedicated sem per DMA, cumulative sems per engine.
        s_cdma = ctx.enter_context(nc.semaphore("s_cdma"))
        s_c2 = ctx.enter_context(nc.semaphore("s_c2"))
        # lane sems: at most ONE in-flight DMA per lane (enforced by the
        # consumer-side slot gating), so per-lane cumulative counts are sound.
        s_xt = [ctx.enter_context(nc.semaphore(f"s_xt{i}"))
                for i in range(XT_LANES)]
        s_b = [ctx.enter_context(nc.semaphore(f"s_b{i}"))
               for i in range(PIECE_LANES)]
        s_m = [ctx.enter_context(nc.semaphore(f"s_m{i}"))
               for i in range(PIECE_LANES)]
        s_od = [ctx.enter_context(nc.semaphore(f"s_od{i}"))
                for i in range(OD_LANES)]
        s_s = ctx.enter_context(nc.semaphore("s_s"))        # ACT s-op done (1/piece)
        s_w = ctx.enter_context(nc.semaphore("s_w"))        # DVE w-op done (1/piece)
        s_mm = ctx.enter_context(nc.semaphore("s_mm"))      # PE group done (1/group)
        s_oc = ctx.enter_context(nc.semaphore("s_oc"))      # ACT out-copy done (1/group)

        w_sb = ctx.enter_context(nc.sbuf_tensor("w_sb", [P, KT, NC], f32r))
        xt_sb = ctx.enter_context(
            nc.sbuf_tensor("xt_sb", [P, SLAB_BUFS, KT, P], f32r))
        b_sb = ctx.enter_context(nc.sbuf_tensor("b_sb", [P, CHUNK_BUFS, NT], f32))
        m_sb = ctx.enter_context(nc.sbuf_tensor("m_sb", [P, CHUNK_BUFS, NT], i32))
        sa_sb = ctx.enter_context(nc.sbuf_tensor("sa_sb", [P, CHUNK_BUFS, NT], f32))
        o_sb = ctx.enter_context(nc.sbuf_tensor("o_sb", [P, OUT_BUFS, NT], f32))
        c_sb = ctx.enter_context(nc.sbuf_tensor("c_sb", [P, 1], f32))
        c2_sb = ctx.enter_context(nc.sbuf_tensor("c2_sb", [P, 1], f32))
        cn_sb = ctx.enter_context(nc.sbuf_tensor("cn_sb", [P, 1], f32))
        ps = [
            ctx.enter_context(nc.psum_tensor(f"ps{i}", [P, NT], f32))
            for i in range(PSB)
        ]

        with nc.Block() as block:

            @block.sync
            def _(sync):
                sync.dma_start(c_sb[:], coeff[:]).then_inc(s_cdma, 16)
                # first slabs (buffers all free)
                for s in range(min(SLAB_BUFS, MS)):
                    sync.dma_start(
                        xt_sb[:, s % SLAB_BUFS], xT3[:, :, s * P:(s + 1) * P]
                    ).then_inc(s_xt[s % XT_LANES], 16)
                # W pieces, k-major
                for j in range(N_PIECES):
                    k, h = j // NH, j % NH
                    if j >= CHUNK_BUFS:
                        # chunk slot reuse: base read by w-op, mask by s-op
                        sync.wait_ge(s_w, j - CHUNK_BUFS + 1)
                        sync.wait_ge(s_s, j - CHUNK_BUFS + 1)
                    sync.dma_start(
                        b_sb[:, j % CHUNK_BUFS],
                        base3[:, k, h * NT:(h + 1) * NT],
                    ).then_inc(s_b[j % PIECE_LANES], 16)
                    sync.dma_start(
                        m_sb[:, j % CHUNK_BUFS],
                        mask3[:, k, h * NT:(h + 1) * NT],
                    ).then_inc(s_m[j % PIECE_LANES], 16)
                # remaining slabs
                for s in range(SLAB_BUFS, MS):
                    # slab slot reuse: strip s-SLAB_BUFS fully consumed by PE
                    sync.wait_ge(s_mm, NH * (s - SLAB_BUFS + 1))
                    sync.dma_start(
                        xt_sb[:, s % SLAB_BUFS], xT3[:, :, s * P:(s + 1) * P]
                    ).then_inc(s_xt[s % XT_LANES], 16)

            @block.scalar
            def _(scalar):
                scalar.wait_ge(s_cdma, 16)
                scalar.activation(c2_sb[:], c_sb[:], Copy, scale=2.0)
                scalar.activation(cn_sb[:], c_sb[:], Copy, scale=-1.0) \
                    .then_inc(s_c2, 1)
                # scale/bias operands are fetched at dispatch; wait for our own
                # writes to drain before the first use
                scalar.wait_ge(s_c2, 1)
                for j in range(N_PIECES):
                    scalar.wait_ge(s_m[j % PIECE_LANES], 16 * (j // PIECE_LANES + 1))
                    if j >= CHUNK_BUFS:
                        # s-chunk slot reuse: previous reader is w-op j-CHUNK_BUFS
                        scalar.wait_ge(s_w, j - CHUNK_BUFS + 1)
                    scalar.activation(
                        sa_sb[:, j % CHUNK_BUFS], m_sb[:, j % CHUNK_BUFS],
                        Identity, scale=c2_sb[:], bias=cn_sb[:],
                    ).then_inc(s_s, 1)
                # PSUM -> SBUF copies
                for g in range(N_GROUPS):
                    scalar.wait_ge(s_mm, g + 1)
                    if g >= OUT_BUFS:
                        gp = g - OUT_BUFS
                        scalar.wait_ge(s_od[gp % OD_LANES],
                                       16 * (gp // OD_LANES + 1))
                    scalar.copy(o_sb[:, g % OUT_BUFS], ps[g % PSB][:]) \
                        .then_inc(s_oc, 1)

            @block.vector
            def _(vector):
                for j in range(N_PIECES):
                    k, h = j // NH, j % NH
                    vector.wait_ge(s_s, j + 1)
                    vector.wait_ge(s_b[j % PIECE_LANES], 16 * (j // PIECE_LANES + 1))
                    vector.tensor_tensor(
                        w_sb[:, k, h * NT:(h + 1) * NT],
                        sa_sb[:, j % CHUNK_BUFS], b_sb[:, j % CHUNK_BUFS],
                        mybir.AluOpType.add,
                    ).then_inc(s_w, 1)

            @block.tensor
            def _(tensor):
                # strip 0: k-major so matmuls chase the W build
                tensor.wait_ge(s_xt[0], 16)  # slab 0 = lane 0, first use
                for k in range(KT):
                    for h in range(NH):
                        tensor.wait_ge(s_w, NH * k + h + 1)
                        mm = tensor.matmul(
                            ps[h][:], xt_sb[:, 0, k, :],
                            w_sb[:, k, h * NT:(h + 1) * NT],
                            start=(k == 0), stop=(k == KT - 1),
                        )
                        if k == KT - 1:
                            # h=0's stop fires before h=1's: group order g=0,1
                            mm.then_inc(s_mm, 1)
                for strip in range(1, MS):
                    tensor.wait_ge(s_xt[strip % XT_LANES],
                                   16 * (strip // XT_LANES + 1))
                    for h in range(NH):
                        g = NH * strip + h
                        if g >= PSB:
                            tensor.wait_ge(s_oc, g - PSB + 1)
                        for k in range(KT):
                            mm = tensor.matmul(
                                ps[g % PSB][:],
                                xt_sb[:, strip % SLAB_BUFS, k, :],
                                w_sb[:, k, h * NT:(h + 1) * NT],
                                start=(k == 0), stop=(k == KT - 1),
                            )
                            if k == KT - 1:
                                mm.then_inc(s_mm, 1)

            @block.gpsimd
            def _(gpsimd):
                for g in range(N_GROUPS):
                    strip, h = g // NH, g % NH
                    gpsimd.wait_ge(s_oc, g + 1)
                    gpsimd.dma_start(
                        out3[:, strip, h * NT:(h + 1) * NT],
                        o_sb[:, g % OUT_BUFS],
                    ).then_inc(s_od[g % OD_LANES], 16)
                for i in range(OD_LANES):
                    cnt = (N_GROUPS - 1 - i) // OD_LANES + 1
                    gpsimd.wait_ge(s_od[i], 16 * cnt)

    return nc


_PROG = None


def kernel(x, base, coeff, mask):
    global _PROG
    if _PROG is None:
        _PROG = _build_program()

    x = np.asarray(x, dtype=np.float32).reshape(ROWS, K)
    base = np.asarray(base, dtype=np.float32)
    mask = np.asarray(mask, dtype=np.int32)
    coeff_np = np.full((P, 1), np.float32(coeff), dtype=np.float32)

    in_maps = []
    shard_ids = []
    for r in range(R_SHARDS):
        xT_r = np.ascontiguousarray(x[r * M:(r + 1) * M, :].T)  # [K, M]
        for c in range(C_SHARDS):
            in_maps.append({
                "xT": xT_r,
                "base": np.ascontiguousarray(base[:, c * NC:(c + 1) * NC]),
                "mask": np.ascontiguousarray(mask[:, c * NC:(c + 1) * NC]),
                "coeff": coeff_np,
            })
            shard_ids.append((r, c))

    res = run_bass_kernel_spmd(_PROG, in_maps, list(range(8))).results

    out = np.empty((ROWS, D_OUT), dtype=np.float32)
    for i, (r, c) in enumerate(shard_ids):
        out[r * M:(r + 1) * M, c * NC:(c + 1) * NC] = res[i]["out"]
    return out.reshape(B, S, D_OUT)


# revision 12
# speedup vs baseline: 1.1634x; 1.1634x over previous
"""BinaryDiff kernel for Trainium2 (8 NeuronCores).

Computes out = x @ base + coeff * (x @ (2*mask - 1)) by folding the two
matmuls into one:  out = x @ W,  W = base + coeff*(2*mask - 1).

Sharding (8 cores = 2 row-groups x 4 col-groups):
  - x rows (B*S = 8192) split in 2 -> each core gets an x^T shard
    [4096 K, 4096 rows], pre-arranged on host in slab-major layout so
    every DMA is contiguous per partition
  - base/mask cols (4096) split in 4 -> per-core shards [4096, 1024]
  - each core computes out shard [4096, 1024]; host concatenates.

On-device per core:
  - W = base + (2c*mask - c) built once into resident SBUF ([128,32,1024]
    fp16) via ACT affine (int32->f32, runtime coeff via scale/bias APs)
    + DVE add (fp32 -> fp16 output).
  - x^T streamed in 32 slabs [128,32,128] (fp32 DMA), converted fp32->fp16
    by DVE; 32 m-strips x 2 n-halves x 32 k-chunks of fp16 matmuls
    (moving dim 512) accumulate in fp32 across 8 PSUM banks.
  - ACT copies PSUM->SBUF (fp32), gpsimd DMAs results out.

Raw bass with manual semaphores. Two hard rules learned on this stack:
  1. Engine datapath instructions may carry at most ONE sync wait, so
     every wait is a standalone wait_ge on the consuming engine.
  2. DMA completions across different HW queues are unordered, so a
     cumulative semaphore over many in-flight DMAs is racy. DMAs use
     per-lane semaphores with at most one outstanding DMA per lane
     (enforced by the consumer-side slot gating). Engine completions
     retire in order, so cumulative per-engine semaphores are sound.
"""
import contextlib

import numpy as np

import concourse.bass as bass
import concourse.mybir as mybir
from concourse.bass_utils import run_bass_kernel_spmd

f32 = mybir.dt.float32
fp16 = mybir.dt.float16
i32 = mybir.dt.int32
Copy = mybir.ActivationFunctionType.Copy
Identity = mybir.ActivationFunctionType.Identity

P = 128
B, S, D_IN, D_OUT = 4, 2048, 4096, 4096
ROWS = B * S                  # 8192
R_SHARDS, C_SHARDS = 2, 4
M = ROWS // R_SHARDS          # 4096 rows per core
NC = D_OUT // C_SHARDS        # 1024 cols per core
K = D_IN                      # 4096 contraction
KT = K // P                   # 32 k-chunks
MS = M // P                   # 32 m-strips
NH = NC // 512                # 2 n-halves
NT = 512
N_PIECES = KT * NH            # 64 W build pieces (k-major: piece j -> k=j//2, h=j%2)
N_GROUPS = MS * NH            # 64 output groups
SLAB_BUFS = 2
CHUNK_BUFS = 3
OUT_BUFS = 4
PSB = 8                       # psum banks in rotation
XT_LANES = 4                  # slab DMA sem lanes (> SLAB_BUFS)
PIECE_LANES = 8               # W piece DMA sem lanes (> CHUNK_BUFS)
OD_LANES = 8                  # out DMA sem lanes (> OUT_BUFS)


def _build_program():
    nc = bass.Bass()
    # xT arrives in slab-major layout: xT_host[s, p, ko, i] = x[s*128+i, ko*128+p]
    # so each slab DMA reads 128 partitions x 16KB fully contiguous.
    xT = nc.declare_dram_parameter("xT", [MS * P, KT * P], f32, isOutput=False)
    base = nc.declare_dram_parameter("base", [K, NC], f32, isOutput=False)
    mask = nc.declare_dram_parameter("mask", [K, NC], i32, isOutput=False)
    coeff = nc.declare_dram_parameter("coeff", [P, 1], f32, isOutput=False)
    out = nc.declare_dram_parameter("out", [M, NC], f32, isOutput=True)

    xT3 = xT.rearrange("(s p) (ko i) -> s p ko i", p=P, i=P)
    base3 = base.rearrange("(ko p) n -> p ko n", p=P)
    mask3 = mask.rearrange("(ko p) n -> p ko n", p=P)
    out3 = out.rearrange("(mo p) n -> p mo n", p=P)

    with contextlib.ExitStack() as ctx:
        s_cdma = ctx.enter_context(nc.semaphore("s_cdma"))
        s_c2 = ctx.enter_context(nc.semaphore("s_c2"))
        s_xt = [ctx.enter_context(nc.semaphore(f"s_xt{i}"))
                for i in range(XT_LANES)]
        s_b = [ctx.enter_context(nc.semaphore(f"s_b{i}"))
               for i in range(PIECE_LANES)]
        s_m = [ctx.enter_context(nc.semaphore(f"s_m{i}"))
               for i in range(PIECE_LANES)]
        s_od = [ctx.enter_context(nc.semaphore(f"s_od{i}"))
                for i in range(OD_LANES)]
        s_s = ctx.enter_context(nc.semaphore("s_s"))      # ACT s-op done (1/piece)
        s_w = ctx.enter_context(nc.semaphore("s_w"))      # DVE w-op done (1/piece)
        s_xtc = ctx.enter_context(nc.semaphore("s_xtc"))  # DVE slab cvt done (1/slab)
        s_mm = ctx.enter_context(nc.semaphore("s_mm"))    # PE group done (1/group)
        s_oc = ctx.enter_context(nc.semaphore("s_oc"))    # ACT out-copy done (1/group)

        w_sb = ctx.enter_context(nc.sbuf_tensor("w_sb", [P, KT, NC], fp16))
        xt_raw = ctx.enter_context(
            nc.sbuf_tensor("xt_raw", [P, SLAB_BUFS, KT, P], f32))
        xt_sb = ctx.enter_context(
            nc.sbuf_tensor("xt_sb", [P, SLAB_BUFS, KT, P], fp16))
        b_sb = ctx.enter_context(nc.sbuf_tensor("b_sb", [P, CHUNK_BUFS, NT], f32))
        m_sb = ctx.enter_context(nc.sbuf_tensor("m_sb", [P, CHUNK_BUFS, NT], i32))
        sa_sb = ctx.enter_context(nc.sbuf_tensor("sa_sb", [P, CHUNK_BUFS, NT], f32))
        o_sb = ctx.enter_context(nc.sbuf_tensor("o_sb", [P, OUT_BUFS, NT], f32))
        c_sb = ctx.enter_context(nc.sbuf_tensor("c_sb", [P, 1], f32))
        c2_sb = ctx.enter_context(nc.sbuf_tensor("c2_sb", [P, 1], f32))
        cn_sb = ctx.enter_context(nc.sbuf_tensor("cn_sb", [P, 1], f32))
        ps = [
            ctx.enter_context(nc.psum_tensor(f"ps{i}", [P, NT], f32))
            for i in range(PSB)
        ]

        with nc.Block() as block:

            @block.sync
            def _(sync):
                sync.dma_start(c_sb[:], coeff[:]).then_inc(s_cdma, 16)
                # first slabs (buffers all free)
                for s in range(min(SLAB_BUFS, MS)):
                    sync.dma_start(
                        xt_raw[:, s % SLAB_BUFS], xT3[s]
                    ).then_inc(s_xt[s % XT_LANES], 16)
                # W pieces, k-major
                for j in range(N_PIECES):
                    k, h = j // NH, j % NH
                    if j >= CHUNK_BUFS:
                        # chunk slot reuse: base read by w-op, mask by s-op
                        sync.wait_ge(s_w, j - CHUNK_BUFS + 1)
                        sync.wait_ge(s_s, j - CHUNK_BUFS + 1)
                    sync.dma_start(
                        b_sb[:, j % CHUNK_BUFS],
                        base3[:, k, h * NT:(h + 1) * NT],
                    ).then_inc(s_b[j % PIECE_LANES], 16)
                    sync.dma_start(
                        m_sb[:, j % CHUNK_BUFS],
                        mask3[:, k, h * NT:(h + 1) * NT],
                    ).then_inc(s_m[j % PIECE_LANES], 16)
                # remaining slabs
                for s in range(SLAB_BUFS, MS):
                    # raw slot reuse: slab s-SLAB_BUFS converted by DVE
                    sync.wait_ge(s_xtc, s - SLAB_BUFS + 1)
                    sync.dma_start(
                        xt_raw[:, s % SLAB_BUFS], xT3[s]
                    ).then_inc(s_xt[s % XT_LANES], 16)

            @block.scalar
            def _(scalar):
                scalar.wait_ge(s_cdma, 16)
                scalar.activation(c2_sb[:], c_sb[:], Copy, scale=2.0)
                scalar.activation(cn_sb[:], c_sb[:], Copy, scale=-1.0) \
                    .then_inc(s_c2, 1)
                # scale/bias operands are fetched at dispatch; wait for our own
                # writes to drain before the first use
                scalar.wait_ge(s_c2, 1)
                for j in range(N_PIECES):
                    scalar.wait_ge(s_m[j % PIECE_LANES],
                                   16 * (j // PIECE_LANES + 1))
                    if j >= CHUNK_BUFS:
                        # s-chunk slot reuse: previous reader is w-op j-CHUNK_BUFS
                        scalar.wait_ge(s_w, j - CHUNK_BUFS + 1)
                    scalar.activation(
                        sa_sb[:, j % CHUNK_BUFS], m_sb[:, j % CHUNK_BUFS],
                        Identity, scale=c2_sb[:], bias=cn_sb[:],
                    ).then_inc(s_s, 1)
                # PSUM -> SBUF copies
                for g in range(N_GROUPS):
                    scalar.wait_ge(s_mm, g + 1)
                    if g >= OUT_BUFS:
                        gp = g - OUT_BUFS
                        scalar.wait_ge(s_od[gp % OD_LANES],
                                       16 * (gp // OD_LANES + 1))
                    scalar.copy(o_sb[:, g % OUT_BUFS], ps[g % PSB][:]) \
                        .then_inc(s_oc, 1)

            @block.vector
            def _(vector):
                def convert_slab(s):
                    vector.wait_ge(s_xt[s % XT_LANES], 16 * (s // XT_LANES + 1))
                    if s >= SLAB_BUFS:
                        # fp16 slot reuse: strip s-SLAB_BUFS fully consumed by PE
                        vector.wait_ge(s_mm, NH * (s - SLAB_BUFS + 1))
                    vector.tensor_copy(
                        xt_sb[:, s % SLAB_BUFS], xt_raw[:, s % SLAB_BUFS]
                    ).then_inc(s_xtc, 1)

                convert_slab(0)
                convert_slab(1)
                for j in range(N_PIECES):
                    k, h = j // NH, j % NH
                    vector.wait_ge(s_s, j + 1)
                    vector.wait_ge(s_b[j % PIECE_LANES],
                                   16 * (j // PIECE_LANES + 1))
                    vector.tensor_tensor(
                        w_sb[:, k, h * NT:(h + 1) * NT],
                        sa_sb[:, j % CHUNK_BUFS], b_sb[:, j % CHUNK_BUFS],
                        mybir.AluOpType.add,
                    ).then_inc(s_w, 1)
                for s in range(SLAB_BUFS, MS):
                    convert_slab(s)

            @block.tensor
            def _(tensor):
                # strip 0: k-major so matmuls chase the W build
                tensor.wait_ge(s_xtc, 1)
                for k in range(KT):
                    for h in range(NH):
                        tensor.wait_ge(s_w, NH * k + h + 1)
                        mm = tensor.matmul(
                            ps[h][:], xt_sb[:, 0, k, :],
                            w_sb[:, k, h * NT:(h + 1) * NT],
                            start=(k == 0), stop=(k == KT - 1),
                        )
                        if k == KT - 1:
                            # h=0's stop fires before h=1's: group order g=0,1
                            mm.then_inc(s_mm, 1)
                for strip in range(1, MS):
                    tensor.wait_ge(s_xtc, strip + 1)
                    for h in range(NH):
                        g = NH * strip + h
                        if g >= PSB:
                            tensor.wait_ge(s_oc, g - PSB + 1)
                        for k in range(KT):
                            mm = tensor.matmul(
                                ps[g % PSB][:],
                                xt_sb[:, strip % SLAB_BUFS, k, :],
                                w_sb[:, k, h * NT:(h + 1) * NT],
                                start=(k == 0), stop=(k == KT - 1),
                            )
                            if k == KT - 1:
                                mm.then_inc(s_mm, 1)

            @block.gpsimd
            def _(gpsimd):
                for g in range(N_GROUPS):
                    strip, h = g // NH, g % NH
                    gpsimd.wait_ge(s_oc, g + 1)
                    gpsimd.dma_start(
                        out3[:, strip, h * NT:(h + 1) * NT],
                        o_sb[:, g % OUT_BUFS],
                    ).then_inc(s_od[g % OD_LANES], 16)
                for i in range(OD_LANES):
                    cnt = (N_GROUPS - 1 - i) // OD_LANES + 1
                    gpsimd.wait_ge(s_od[i], 16 * cnt)

    return nc


_PROG = None


def kernel(x, base, coeff, mask):
    global _PROG
    if _PROG is None:
        _PROG = _build_program()

    x = np.asarray(x, dtype=np.float32).reshape(ROWS, K)
    base = np.asarray(base, dtype=np.float32)
    mask = np.asarray(mask, dtype=np.int32)
    coeff_np = np.full((P, 1), np.float32(coeff), dtype=np.float32)

    in_maps = []
    shard_ids = []
    for r in range(R_SHARDS):
        x_r = x[r * M:(r + 1) * M, :]
        # slab-major: [s, p, ko, i] = x_r[s*128+i, ko*128+p]
        xT_r = np.ascontiguousarray(
            x_r.reshape(MS, P, KT, P).transpose(0, 3, 2, 1)
        ).reshape(MS * P, KT * P)
        for c in range(C_SHARDS):
            in_maps.append({
                "xT": xT_r,
                "base": np.ascontiguousarray(base[:, c * NC:(c + 1) * NC]),
                "mask": np.ascontiguousarray(mask[:, c * NC:(c + 1) * NC]),
                "coeff": coeff_np,
            })
            shard_ids.append((r, c))

    res = run_bass_kernel_spmd(_PROG, in_maps, list(range(8))).results

    out = np.empty((ROWS, D_OUT), dtype=np.float32)
    for i, (r, c) in enumerate(shard_ids):
        out[r * M:(r + 1) * M, c * NC:(c + 1) * NC] = res[i]["out"]
    return out.reshape(B, S, D_OUT)


# revision 13
# speedup vs baseline: 2.3390x; 2.0106x over previous
"""BinaryDiff kernel for Trainium2 (8 NeuronCores).

Computes out = x @ base + coeff * (x @ (2*mask - 1)) by folding the two
matmuls into one:  out = x @ W,  W = base + coeff*(2*mask - 1).

Sharding (8 cores = 2 row-groups x 4 col-groups):
  - x rows (B*S = 8192) split in 2 -> each core gets an x^T shard
    [4096 K, 4096 rows], pre-arranged on host in slab-major layout so
    every DMA is contiguous per partition
  - base/mask cols (4096) split in 4 -> per-core shards [4096, 1024]
  - each core computes out shard [4096, 1024]; host concatenates.

On-device per core:
  - W = base + (2c*mask - c) built once into resident SBUF ([128,32,1024]
    fp16) via ACT affine (int32->f32, runtime coeff via scale/bias APs)
    + DVE add (fp32 -> fp16 output).
  - x^T streamed in 32 slabs [128,32,128] (fp32 DMA), converted fp32->fp16
    by DVE; 32 m-strips x 2 n-halves x 32 k-chunks of fp16 matmuls
    (moving dim 512) accumulate in fp32 across 8 PSUM banks.
  - ACT copies PSUM->SBUF (fp32), gpsimd DMAs results out.

Raw bass with manual semaphores. Two hard rules learned on this stack:
  1. Engine datapath instructions may carry at most ONE sync wait, so
     every wait is a standalone wait_ge on the consuming engine.
  2. DMA completions across different HW queues are unordered, so a
     cumulative semaphore over many in-flight DMAs is racy. DMAs use
     per-lane semaphores with at most one outstanding DMA per lane
     (enforced by the consumer-side slot gating). Engine completions
     retire in order, so cumulative per-engine semaphores are sound.
"""
import contextlib

import numpy as np

import concourse.bass as bass
import concourse.mybir as mybir
from concourse.bass_utils import run_bass_kernel_spmd

f32 = mybir.dt.float32
fp16 = mybir.dt.float16
i32 = mybir.dt.int32
Copy = mybir.ActivationFunctionType.Copy
Identity = mybir.ActivationFunctionType.Identity

P = 128
B, S, D_IN, D_OUT = 4, 2048, 4096, 4096
ROWS = B * S                  # 8192
R_SHARDS, C_SHARDS = 2, 4
M = ROWS // R_SHARDS          # 4096 rows per core
NC = D_OUT // C_SHARDS        # 1024 cols per core
K = D_IN                      # 4096 contraction
KT = K // P                   # 32 k-chunks
MS = M // P                   # 32 m-strips
NH = NC // 512                # 2 n-halves
NT = 512
N_PIECES = KT * NH            # 64 W build pieces (k-major: piece j -> k=j//2, h=j%2)
N_GROUPS = MS * NH            # 64 output groups
SLAB_BUFS = 2
CHUNK_BUFS = 3
OUT_BUFS = 4
PSB = 8                       # psum banks in rotation
XT_LANES = 4                  # slab DMA sem lanes (> SLAB_BUFS)
PIECE_LANES = 8               # W piece DMA sem lanes (> CHUNK_BUFS)
OD_LANES = 8                  # out DMA sem lanes (> OUT_BUFS)


def _build_program(reps=1):
    """reps > 1 repeats the whole pipeline inside one NEFF (for timing:
    T(reps=a) - T(reps=b) isolates (a-b) kernel bodies from dispatch
    overhead). Functionally identical output (each rep overwrites out)."""
    nc = bass.Bass()
    # xT arrives in slab-major layout: xT_host[s, p, ko, i] = x[s*128+i, ko*128+p]
    # so each slab DMA reads 128 partitions x 16KB fully contiguous.
    xT = nc.declare_dram_parameter("xT", [MS * P, KT * P], f32, isOutput=False)
    base = nc.declare_dram_parameter("base", [K, NC], f32, isOutput=False)
    mask = nc.declare_dram_parameter("mask", [K, NC], i32, isOutput=False)
    coeff = nc.declare_dram_parameter("coeff", [P, 1], f32, isOutput=False)
    out = nc.declare_dram_parameter("out", [M, NC], f32, isOutput=True)

    xT3 = xT.rearrange("(s p) (ko i) -> s p ko i", p=P, i=P)
    base3 = base.rearrange("(ko p) n -> p ko n", p=P)
    mask3 = mask.rearrange("(ko p) n -> p ko n", p=P)
    out3 = out.rearrange("(mo p) n -> p mo n", p=P)

    with contextlib.ExitStack() as ctx:
        s_cdma = ctx.enter_context(nc.semaphore("s_cdma"))
        s_c2 = ctx.enter_context(nc.semaphore("s_c2"))
        s_xt = [ctx.enter_context(nc.semaphore(f"s_xt{i}"))
                for i in range(XT_LANES)]
        s_b = [ctx.enter_context(nc.semaphore(f"s_b{i}"))
               for i in range(PIECE_LANES)]
        s_m = [ctx.enter_context(nc.semaphore(f"s_m{i}"))
               for i in range(PIECE_LANES)]
        s_od = [ctx.enter_context(nc.semaphore(f"s_od{i}"))
                for i in range(OD_LANES)]
        s_s = ctx.enter_context(nc.semaphore("s_s"))      # ACT s-op done (1/piece)
        s_w = ctx.enter_context(nc.semaphore("s_w"))      # DVE w-op done (1/piece)
        s_xtc = ctx.enter_context(nc.semaphore("s_xtc"))  # DVE slab cvt done (1/slab)
        s_mm = ctx.enter_context(nc.semaphore("s_mm"))    # PE group done (1/group)
        s_oc = ctx.enter_context(nc.semaphore("s_oc"))    # ACT out-copy done (1/group)

        w_sb = ctx.enter_context(nc.sbuf_tensor("w_sb", [P, KT, NC], fp16))
        xt_raw = ctx.enter_context(
            nc.sbuf_tensor("xt_raw", [P, SLAB_BUFS, KT, P], f32))
        xt_sb = ctx.enter_context(
            nc.sbuf_tensor("xt_sb", [P, SLAB_BUFS, KT, P], fp16))
        b_sb = ctx.enter_context(nc.sbuf_tensor("b_sb", [P, CHUNK_BUFS, NT], f32))
        m_sb = ctx.enter_context(nc.sbuf_tensor("m_sb", [P, CHUNK_BUFS, NT], i32))
        sa_sb = ctx.enter_context(nc.sbuf_tensor("sa_sb", [P, CHUNK_BUFS, NT], f32))
        o_sb = ctx.enter_context(nc.sbuf_tensor("o_sb", [P, OUT_BUFS, NT], f32))
        c_sb = ctx.enter_context(nc.sbuf_tensor("c_sb", [P, 1], f32))
        c2_sb = ctx.enter_context(nc.sbuf_tensor("c2_sb", [P, 1], f32))
        cn_sb = ctx.enter_context(nc.sbuf_tensor("cn_sb", [P, 1], f32))
        ps = [
            ctx.enter_context(nc.psum_tensor(f"ps{i}", [P, NT], f32))
            for i in range(PSB)
        ]

        with nc.Block() as block:

            @block.sync
            def _(sync):
                sync.dma_start(c_sb[:], coeff[:]).then_inc(s_cdma, 16)
                for it in range(reps):
                    bW = it * N_PIECES          # s_s/s_w base
                    bX = it * MS                # slab count base
                    bL = it * (MS // XT_LANES) * 16   # per-lane slab base
                    bP = it * (N_PIECES // PIECE_LANES) * 16
                    # first slabs of this rep (slot free once cvt of s-2 done)
                    for s in range(min(SLAB_BUFS, MS)):
                        if bX + s >= SLAB_BUFS:
                            sync.wait_ge(s_xtc, bX + s - SLAB_BUFS + 1)
                        sync.dma_start(
                            xt_raw[:, s % SLAB_BUFS], xT3[s]
                        ).then_inc(s_xt[s % XT_LANES], 16)
                    # W pieces, k-major
                    for j in range(N_PIECES):
                        k, h = j // NH, j % NH
                        if bW + j >= CHUNK_BUFS:
                            sync.wait_ge(s_w, bW + j - CHUNK_BUFS + 1)
                            sync.wait_ge(s_s, bW + j - CHUNK_BUFS + 1)
                        sync.dma_start(
                            b_sb[:, j % CHUNK_BUFS],
                            base3[:, k, h * NT:(h + 1) * NT],
                        ).then_inc(s_b[j % PIECE_LANES], 16)
                        sync.dma_start(
                            m_sb[:, j % CHUNK_BUFS],
                            mask3[:, k, h * NT:(h + 1) * NT],
                        ).then_inc(s_m[j % PIECE_LANES], 16)
                    # remaining slabs
                    for s in range(SLAB_BUFS, MS):
                        sync.wait_ge(s_xtc, bX + s - SLAB_BUFS + 1)
                        sync.dma_start(
                            xt_raw[:, s % SLAB_BUFS], xT3[s]
                        ).then_inc(s_xt[s % XT_LANES], 16)

            @block.scalar
            def _(scalar):
                scalar.wait_ge(s_cdma, 16)
                scalar.activation(c2_sb[:], c_sb[:], Copy, scale=2.0)
                scalar.activation(cn_sb[:], c_sb[:], Copy, scale=-1.0) \
                    .then_inc(s_c2, 1)
                # scale/bias operands are fetched at dispatch; wait for our own
                # writes to drain before the first use
                scalar.wait_ge(s_c2, 1)
                for it in range(reps):
                    bW = it * N_PIECES
                    bG = it * N_GROUPS
                    bP = it * (N_PIECES // PIECE_LANES) * 16
                    bO = it * (N_GROUPS // OD_LANES) * 16
                    for j in range(N_PIECES):
                        scalar.wait_ge(s_m[j % PIECE_LANES],
                                       bP + 16 * (j // PIECE_LANES + 1))
                        if bW + j >= CHUNK_BUFS:
                            scalar.wait_ge(s_w, bW + j - CHUNK_BUFS + 1)
                        scalar.activation(
                            sa_sb[:, j % CHUNK_BUFS], m_sb[:, j % CHUNK_BUFS],
                            Identity, scale=c2_sb[:], bias=cn_sb[:],
                        ).then_inc(s_s, 1)
                    # PSUM -> SBUF copies
                    for g in range(N_GROUPS):
                        scalar.wait_ge(s_mm, bG + g + 1)
                        if bG + g >= OUT_BUFS:
                            gp = bG + g - OUT_BUFS
                            scalar.wait_ge(s_od[gp % OD_LANES],
                                           16 * (gp // OD_LANES + 1))
                        scalar.copy(o_sb[:, g % OUT_BUFS], ps[g % PSB][:]) \
                            .then_inc(s_oc, 1)

            @block.vector
            def _(vector):
                for it in range(reps):
                    bW = it * N_PIECES
                    bX = it * MS
                    bL = it * (MS // XT_LANES) * 16
                    bP = it * (N_PIECES // PIECE_LANES) * 16

                    def convert_slab(s, bX=bX, bL=bL, it=it):
                        vector.wait_ge(s_xt[s % XT_LANES],
                                       bL + 16 * (s // XT_LANES + 1))
                        if bX + s >= SLAB_BUFS:
                            # fp16 slot reuse: strip s-SLAB_BUFS consumed by PE
                            vector.wait_ge(s_mm, NH * (bX + s - SLAB_BUFS + 1))
                        vector.tensor_copy(
                            xt_sb[:, s % SLAB_BUFS], xt_raw[:, s % SLAB_BUFS]
                        ).then_inc(s_xtc, 1)

                    convert_slab(0)
                    convert_slab(1)
                    for j in range(N_PIECES):
                        k, h = j // NH, j % NH
                        vector.wait_ge(s_s, bW + j + 1)
                        vector.wait_ge(s_b[j % PIECE_LANES],
                                       bP + 16 * (j // PIECE_LANES + 1))
                        vector.tensor_tensor(
                            w_sb[:, k, h * NT:(h + 1) * NT],
                            sa_sb[:, j % CHUNK_BUFS], b_sb[:, j % CHUNK_BUFS],
                            mybir.AluOpType.add,
                        ).then_inc(s_w, 1)
                    for s in range(SLAB_BUFS, MS):
                        convert_slab(s)

            @block.tensor
            def _(tensor):
                for it in range(reps):
                    bW = it * N_PIECES
                    bX = it * MS
                    bG = it * N_GROUPS
                    # strip 0: k-major so matmuls chase the W build
                    tensor.wait_ge(s_xtc, bX + 1)
                    for k in range(KT):
                        for h in range(NH):
                            g = bG + h
                            if g >= PSB:
                                pass  # handled via s_oc wait below for k==0
                            tensor.wait_ge(s_w, bW + NH * k + h + 1)
                            if k == 0 and g >= PSB:
                                tensor.wait_ge(s_oc, g - PSB + 1)
                            mm = tensor.matmul(
                                ps[g % PSB][:], xt_sb[:, 0, k, :],
                                w_sb[:, k, h * NT:(h + 1) * NT],
                                start=(k == 0), stop=(k == KT - 1),
                            )
                            if k == KT - 1:
                                # h=0 stop fires before h=1: group order
                                mm.then_inc(s_mm, 1)
                    for strip in range(1, MS):
                        tensor.wait_ge(s_xtc, bX + strip + 1)
                        for h in range(NH):
                            g = bG + NH * strip + h
                            if g >= PSB:
                                tensor.wait_ge(s_oc, g - PSB + 1)
                            for k in range(KT):
                                mm = tensor.matmul(
                                    ps[g % PSB][:],
                                    xt_sb[:, strip % SLAB_BUFS, k, :],
                                    w_sb[:, k, h * NT:(h + 1) * NT],
                                    start=(k == 0), stop=(k == KT - 1),
                                )
                                if k == KT - 1:
                                    mm.then_inc(s_mm, 1)

            @block.gpsimd
            def _(gpsimd):
                for it in range(reps):
                    bG = it * N_GROUPS
                    for g in range(N_GROUPS):
                        strip, h = g // NH, g % NH
                        gpsimd.wait_ge(s_oc, bG + g + 1)
                        gpsimd.dma_start(
                            out3[:, strip, h * NT:(h + 1) * NT],
                            o_sb[:, g % OUT_BUFS],
                        ).then_inc(s_od[g % OD_LANES], 16)
                for i in range(OD_LANES):
                    cnt = (reps * N_GROUPS - 1 - i) // OD_LANES + 1
                    gpsimd.wait_ge(s_od[i], 16 * cnt)

    return nc


_PROG = None


def kernel(x, base, coeff, mask):
    global _PROG
    if _PROG is None:
        _PROG = _build_program()

    x = np.asarray(x, dtype=np.float32).reshape(ROWS, K)
    base = np.asarray(base, dtype=np.float32)
    mask = np.asarray(mask, dtype=np.int32)
    coeff_np = np.full((P, 1), np.float32(coeff), dtype=np.float32)

    in_maps = []
    shard_ids = []
    for r in range(R_SHARDS):
        x_r = x[r * M:(r + 1) * M, :]
        # slab-major: [s, p, ko, i] = x_r[s*128+i, ko*128+p]
        xT_r = np.ascontiguousarray(
            x_r.reshape(MS, P, KT, P).transpose(0, 3, 2, 1)
        ).reshape(MS * P, KT * P)
        for c in range(C_SHARDS):
            in_maps.append({
                "xT": xT_r,
                "base": np.ascontiguousarray(base[:, c * NC:(c + 1) * NC]),
                "mask": np.ascontiguousarray(mask[:, c * NC:(c + 1) * NC]),
                "coeff": coeff_np,
            })
            shard_ids.append((r, c))

    res = run_bass_kernel_spmd(_PROG, in_maps, list(range(8))).results

    out = np.empty((ROWS, D_OUT), dtype=np.float32)
    for i, (r, c) in enumerate(shard_ids):
        out[r * M:(r + 1) * M, c * NC:(c + 1) * NC] = res[i]["out"]
    return out.reshape(B, S, D_OUT)


# revision 15
# speedup vs baseline: 2.3908x; 1.0221x over previous
"""BinaryDiff kernel for Trainium2 (8 NeuronCores).

Computes out = x @ base + coeff * (x @ (2*mask - 1)) by folding the two
matmuls into one:  out = x @ W,  W = base + coeff*(2*mask - 1).

Sharding (8 cores = 2 row-groups x 4 col-groups):
  - x rows (B*S = 8192) split in 2 -> each core gets an x^T shard
    [4096 K, 4096 rows], pre-arranged on host in slab-major layout so
    every DMA is contiguous per partition
  - base/mask cols (4096) split in 4 -> per-core shards [4096, 1024]
  - each core computes out shard [4096, 1024]; host concatenates.

On-device per core:
  - W = base + (2c*mask - c) built once into resident SBUF ([128,32,1024]
    fp16) via ACT affine (int32->f32, runtime coeff via scale/bias APs)
    + DVE add (fp32 -> fp16 output).
  - x^T streamed in 32 slabs [128,32,128] (fp32 DMA), converted fp32->fp16
    by DVE; 32 m-strips x 2 n-halves x 32 k-chunks of fp16 matmuls
    (moving dim 512) accumulate in fp32 across 8 PSUM banks.
  - ACT copies PSUM->SBUF (fp32), gpsimd DMAs results out.

Raw bass with manual semaphores. Two hard rules learned on this stack:
  1. Engine datapath instructions may carry at most ONE sync wait, so
     every wait is a standalone wait_ge on the consuming engine.
  2. DMA completions across different HW queues are unordered, so a
     cumulative semaphore over many in-flight DMAs is racy. DMAs use
     per-lane semaphores with at most one outstanding DMA per lane
     (enforced by the consumer-side slot gating). Engine completions
     retire in order, so cumulative per-engine semaphores are sound.
"""
import contextlib

import numpy as np

import concourse.bass as bass
import concourse.mybir as mybir
from concourse.bass_utils import run_bass_kernel_spmd

f32 = mybir.dt.float32
fp16 = mybir.dt.float16
i32 = mybir.dt.int32
Copy = mybir.ActivationFunctionType.Copy
Identity = mybir.ActivationFunctionType.Identity

P = 128
B, S, D_IN, D_OUT = 4, 2048, 4096, 4096
ROWS = B * S                  # 8192
R_SHARDS, C_SHARDS = 2, 4
M = ROWS // R_SHARDS          # 4096 rows per core
NC = D_OUT // C_SHARDS        # 1024 cols per core
K = D_IN                      # 4096 contraction
KT = K // P                   # 32 k-chunks
MS = M // P                   # 32 m-strips
NH = NC // 512                # 2 n-halves
NT = 512
N_PIECES = KT                 # 32 W build pieces (one full-width [128,1024] per k)
N_GROUPS = MS * NH            # 64 output groups
SLAB_BUFS = 2
CHUNK_BUFS = 4
OUT_BUFS = 4
PSB = 8                       # psum banks in rotation
XT_LANES = 4                  # slab DMA sem lanes (> SLAB_BUFS)
PIECE_LANES = 8               # W piece DMA sem lanes (> CHUNK_BUFS)
OD_LANES = 8                  # out DMA sem lanes (> OUT_BUFS)


def _build_program(reps=1):
    """reps > 1 repeats the whole pipeline inside one NEFF (for timing:
    T(reps=a) - T(reps=b) isolates (a-b) kernel bodies from dispatch
    overhead). Functionally identical output (each rep overwrites out)."""
    nc = bass.Bass()
    # xT arrives in slab-major layout: xT_host[s, p, ko, i] = x[s*128+i, ko*128+p]
    # so each slab DMA reads 128 partitions x 16KB fully contiguous.
    xT = nc.declare_dram_parameter("xT", [MS * P, KT * P], f32, isOutput=False)
    base = nc.declare_dram_parameter("base", [K, NC], f32, isOutput=False)
    mask = nc.declare_dram_parameter("mask", [K, NC], i32, isOutput=False)
    coeff = nc.declare_dram_parameter("coeff", [P, 1], f32, isOutput=False)
    out = nc.declare_dram_parameter("out", [M, NC], f32, isOutput=True)

    xT3 = xT.rearrange("(s p) (ko i) -> s p ko i", p=P, i=P)
    base3 = base.rearrange("(ko p) n -> p ko n", p=P)
    mask3 = mask.rearrange("(ko p) n -> p ko n", p=P)
    out3 = out.rearrange("(mo p) n -> p mo n", p=P)

    with contextlib.ExitStack() as ctx:
        s_cdma = ctx.enter_context(nc.semaphore("s_cdma"))
        s_c2 = ctx.enter_context(nc.semaphore("s_c2"))
        s_xt = [ctx.enter_context(nc.semaphore(f"s_xt{i}"))
                for i in range(XT_LANES)]
        s_b = [ctx.enter_context(nc.semaphore(f"s_b{i}"))
               for i in range(PIECE_LANES)]
        s_m = [ctx.enter_context(nc.semaphore(f"s_m{i}"))
               for i in range(PIECE_LANES)]
        s_od = [ctx.enter_context(nc.semaphore(f"s_od{i}"))
                for i in range(OD_LANES)]
        s_s = ctx.enter_context(nc.semaphore("s_s"))      # ACT s-op done (1/piece)
        s_w = ctx.enter_context(nc.semaphore("s_w"))      # DVE w-op done (1/piece)
        s_xtc = ctx.enter_context(nc.semaphore("s_xtc"))  # DVE slab cvt done (1/slab)
        s_mm = ctx.enter_context(nc.semaphore("s_mm"))    # PE group done (1/group)
        s_oc = ctx.enter_context(nc.semaphore("s_oc"))    # ACT out-copy done (1/group)

        w_sb = ctx.enter_context(nc.sbuf_tensor("w_sb", [P, KT, NC], fp16))
        xt_raw = ctx.enter_context(
            nc.sbuf_tensor("xt_raw", [P, SLAB_BUFS, KT, P], f32))
        xt_sb = ctx.enter_context(
            nc.sbuf_tensor("xt_sb", [P, SLAB_BUFS, KT, P], fp16))
        b_sb = ctx.enter_context(nc.sbuf_tensor("b_sb", [P, CHUNK_BUFS, NC], f32))
        m_sb = ctx.enter_context(nc.sbuf_tensor("m_sb", [P, CHUNK_BUFS, NC], i32))
        sa_sb = ctx.enter_context(nc.sbuf_tensor("sa_sb", [P, CHUNK_BUFS, NC], f32))
        o_sb = ctx.enter_context(nc.sbuf_tensor("o_sb", [P, OUT_BUFS, NT], f32))
        c_sb = ctx.enter_context(nc.sbuf_tensor("c_sb", [P, 1], f32))
        c2_sb = ctx.enter_context(nc.sbuf_tensor("c2_sb", [P, 1], f32))
        cn_sb = ctx.enter_context(nc.sbuf_tensor("cn_sb", [P, 1], f32))
        ps = [
            ctx.enter_context(nc.psum_tensor(f"ps{i}", [P, NT], f32))
            for i in range(PSB)
        ]

        with nc.Block() as block:

            @block.sync
            def _(sync):
                sync.dma_start(c_sb[:], coeff[:]).then_inc(s_cdma, 16)
                for it in range(reps):
                    bW = it * N_PIECES          # s_s/s_w base
                    bX = it * MS                # slab count base
                    bL = it * (MS // XT_LANES) * 16   # per-lane slab base
                    bP = it * (N_PIECES // PIECE_LANES) * 16
                    if it > 0:
                        # serialize rep boundaries so per-body timing equals a
                        # single-shot run (also keeps w_sb write/read ordered)
                        sync.wait_ge(s_oc, it * N_GROUPS)
                    # first slabs of this rep (slot free once cvt of s-2 done)
                    for s in range(min(SLAB_BUFS, MS)):
                        if bX + s >= SLAB_BUFS:
                            sync.wait_ge(s_xtc, bX + s - SLAB_BUFS + 1)
                        sync.dma_start(
                            xt_raw[:, s % SLAB_BUFS], xT3[s]
                        ).then_inc(s_xt[s % XT_LANES], 16)
                    # W pieces: one full-width [128, NC] piece per k
                    for j in range(N_PIECES):
                        if bW + j >= CHUNK_BUFS:
                            sync.wait_ge(s_w, bW + j - CHUNK_BUFS + 1)
                            sync.wait_ge(s_s, bW + j - CHUNK_BUFS + 1)
                        sync.dma_start(
                            b_sb[:, j % CHUNK_BUFS], base3[:, j],
                        ).then_inc(s_b[j % PIECE_LANES], 16)
                        sync.dma_start(
                            m_sb[:, j % CHUNK_BUFS], mask3[:, j],
                        ).then_inc(s_m[j % PIECE_LANES], 16)
                    # remaining slabs
                    for s in range(SLAB_BUFS, MS):
                        sync.wait_ge(s_xtc, bX + s - SLAB_BUFS + 1)
                        sync.dma_start(
                            xt_raw[:, s % SLAB_BUFS], xT3[s]
                        ).then_inc(s_xt[s % XT_LANES], 16)

            @block.scalar
            def _(scalar):
                scalar.wait_ge(s_cdma, 16)
                scalar.activation(c2_sb[:], c_sb[:], Copy, scale=2.0)
                scalar.activation(cn_sb[:], c_sb[:], Copy, scale=-1.0) \
                    .then_inc(s_c2, 1)
                # scale/bias operands are fetched at dispatch; wait for our own
                # writes to drain before the first use
                scalar.wait_ge(s_c2, 1)
                for it in range(reps):
                    bW = it * N_PIECES
                    bG = it * N_GROUPS
                    bP = it * (N_PIECES // PIECE_LANES) * 16
                    bO = it * (N_GROUPS // OD_LANES) * 16
                    for j in range(N_PIECES):
                        scalar.wait_ge(s_m[j % PIECE_LANES],
                                       bP + 16 * (j // PIECE_LANES + 1))
                        if bW + j >= CHUNK_BUFS:
                            scalar.wait_ge(s_w, bW + j - CHUNK_BUFS + 1)
                        scalar.activation(
                            sa_sb[:, j % CHUNK_BUFS], m_sb[:, j % CHUNK_BUFS],
                            Identity, scale=c2_sb[:], bias=cn_sb[:],
                        ).then_inc(s_s, 1)
                    # PSUM -> SBUF copies
                    for g in range(N_GROUPS):
                        scalar.wait_ge(s_mm, bG + g + 1)
                        if bG + g >= OUT_BUFS:
                            gp = bG + g - OUT_BUFS
                            scalar.wait_ge(s_od[gp % OD_LANES],
                                           16 * (gp // OD_LANES + 1))
                        scalar.copy(o_sb[:, g % OUT_BUFS], ps[g % PSB][:]) \
                            .then_inc(s_oc, 1)

            @block.vector
            def _(vector):
                for it in range(reps):
                    bW = it * N_PIECES
                    bX = it * MS
                    bL = it * (MS // XT_LANES) * 16
                    bP = it * (N_PIECES // PIECE_LANES) * 16

                    def convert_slab(s, bX=bX, bL=bL, it=it):
                        vector.wait_ge(s_xt[s % XT_LANES],
                                       bL + 16 * (s // XT_LANES + 1))
                        if bX + s >= SLAB_BUFS:
                            # fp16 slot reuse: strip s-SLAB_BUFS consumed by PE
                            vector.wait_ge(s_mm, NH * (bX + s - SLAB_BUFS + 1))
                        vector.tensor_copy(
                            xt_sb[:, s % SLAB_BUFS], xt_raw[:, s % SLAB_BUFS]
                        ).then_inc(s_xtc, 1)

                    convert_slab(0)
                    convert_slab(1)
                    for j in range(N_PIECES):
                        vector.wait_ge(s_s, bW + j + 1)
                        vector.wait_ge(s_b[j % PIECE_LANES],
                                       bP + 16 * (j // PIECE_LANES + 1))
                        vector.tensor_tensor(
                            w_sb[:, j, :],
                            sa_sb[:, j % CHUNK_BUFS], b_sb[:, j % CHUNK_BUFS],
                            mybir.AluOpType.add,
                        ).then_inc(s_w, 1)
                    for s in range(SLAB_BUFS, MS):
                        convert_slab(s)

            @block.tensor
            def _(tensor):
                for it in range(reps):
                    bW = it * N_PIECES
                    bX = it * MS
                    bG = it * N_GROUPS
                    # strip 0: k-major so matmuls chase the W build
                    tensor.wait_ge(s_xtc, bX + 1)
                    for k in range(KT):
                        tensor.wait_ge(s_w, bW + k + 1)
                        for h in range(NH):
                            g = bG + h
                            if k == 0 and g >= PSB:
                                tensor.wait_ge(s_oc, g - PSB + 1)
                            mm = tensor.matmul(
                                ps[g % PSB][:], xt_sb[:, 0, k, :],
                                w_sb[:, k, h * NT:(h + 1) * NT],
                                start=(k == 0), stop=(k == KT - 1),
                            )
                            if k == KT - 1:
                                # h=0 stop fires before h=1: group order
                                mm.then_inc(s_mm, 1)
                    for strip in range(1, MS):
                        tensor.wait_ge(s_xtc, bX + strip + 1)
                        for h in range(NH):
                            g = bG + NH * strip + h
                            if g >= PSB:
                                tensor.wait_ge(s_oc, g - PSB + 1)
                            for k in range(KT):
                                mm = tensor.matmul(
                                    ps[g % PSB][:],
                                    xt_sb[:, strip % SLAB_BUFS, k, :],
                                    w_sb[:, k, h * NT:(h + 1) * NT],
                                    start=(k == 0), stop=(k == KT - 1),
                                )
                                if k == KT - 1:
                                    mm.then_inc(s_mm, 1)

            @block.gpsimd
            def _(gpsimd):
                for it in range(reps):
                    bG = it * N_GROUPS
                    for g in range(N_GROUPS):
                        strip, h = g // NH, g % NH
                        gpsimd.wait_ge(s_oc, bG + g + 1)
                        gpsimd.dma_start(
                            out3[:, strip, h * NT:(h + 1) * NT],
                            o_sb[:, g % OUT_BUFS],
                        ).then_inc(s_od[g % OD_LANES], 16)
                for i in range(OD_LANES):
                    cnt = (reps * N_GROUPS - 1 - i) // OD_LANES + 1
                    gpsimd.wait_ge(s_od[i], 16 * cnt)

    return nc


_PROG = None


def kernel(x, base, coeff, mask):
    global _PROG
    if _PROG is None:
        _PROG = _build_program()

    x = np.asarray(x, dtype=np.float32).reshape(ROWS, K)
    base = np.asarray(base, dtype=np.float32)
    mask = np.asarray(mask, dtype=np.int32)
    coeff_np = np.full((P, 1), np.float32(coeff), dtype=np.float32)

    in_maps = []
    shard_ids = []
    for r in range(R_SHARDS):
        x_r = x[r * M:(r + 1) * M, :]
        # slab-major: [s, p, ko, i] = x_r[s*128+i, ko*128+p]
        xT_r = np.ascontiguousarray(
            x_r.reshape(MS, P, KT, P).transpose(0, 3, 2, 1)
        ).reshape(MS * P, KT * P)
        for c in range(C_SHARDS):
            in_maps.append({
                "xT": xT_r,
                "base": np.ascontiguousarray(base[:, c * NC:(c + 1) * NC]),
                "mask": np.ascontiguousarray(mask[:, c * NC:(c + 1) * NC]),
                "coeff": coeff_np,
            })
            shard_ids.append((r, c))

    res = run_bass_kernel_spmd(_PROG, in_maps, list(range(8))).results

    out = np.empty((ROWS, D_OUT), dtype=np.float32)
    for i, (r, c) in enumerate(shard_ids):
        out[r * M:(r + 1) * M, c * NC:(c + 1) * NC] = res[i]["out"]
    return out.reshape(B, S, D_OUT)


# revision 16
# speedup vs baseline: 2.8523x; 1.1930x over previous
"""BinaryDiff kernel for Trainium2 (8 NeuronCores).

Computes out = x @ base + coeff * (x @ (2*mask - 1)) by folding the two
matmuls into one:  out = x @ W,  W = base + coeff*(2*mask - 1).

Sharding (8 cores = 2 row-groups x 4 col-groups):
  - x rows (B*S = 8192) split in 2 -> each core gets an x^T shard
    [4096 K, 4096 rows], pre-arranged on host in slab-major layout so
    every DMA is contiguous per partition
  - base/mask cols (4096) split in 4 -> per-core shards [4096, 1024]
  - each core computes out shard [4096, 1024]; host concatenates.

On-device per core:
  - W = base + (2c*mask - c) built once into resident SBUF ([128,32,1024]
    fp16) via ACT affine (int32->f32, runtime coeff via scale/bias APs)
    + DVE add (fp32 -> fp16 output).
  - x^T streamed in 32 slabs [128,32,128] (fp32 DMA), converted fp32->fp16
    by DVE; 32 m-strips x 2 n-halves x 32 k-chunks of fp16 matmuls
    (moving dim 512) accumulate in fp32 across 8 PSUM banks.
  - ACT copies PSUM->SBUF (fp32), gpsimd DMAs results out.

Raw bass with manual semaphores. Two hard rules learned on this stack:
  1. Engine datapath instructions may carry at most ONE sync wait, so
     every wait is a standalone wait_ge on the consuming engine.
  2. DMA completions across different HW queues are unordered, so a
     cumulative semaphore over many in-flight DMAs is racy. DMAs use
     per-lane semaphores with at most one outstanding DMA per lane
     (enforced by the consumer-side slot gating). Engine completions
     retire in order, so cumulative per-engine semaphores are sound.
"""
import contextlib

import numpy as np

import concourse.bass as bass
import concourse.mybir as mybir
from concourse.bass_utils import run_bass_kernel_spmd

f32 = mybir.dt.float32
fp16 = mybir.dt.float16
i32 = mybir.dt.int32
Copy = mybir.ActivationFunctionType.Copy
Identity = mybir.ActivationFunctionType.Identity

P = 128
B, S, D_IN, D_OUT = 4, 2048, 4096, 4096
ROWS = B * S                  # 8192
R_SHARDS, C_SHARDS = 2, 4
M = ROWS // R_SHARDS          # 4096 rows per core
NC = D_OUT // C_SHARDS        # 1024 cols per core
K = D_IN                      # 4096 contraction
KT = K // P                   # 32 k-chunks
MS = M // P                   # 32 m-strips
NH = NC // 512                # 2 n-halves
NT = 512
N_PIECES = KT                 # 32 W build pieces (one full-width [128,1024] per k)
N_GROUPS = MS * NH            # 64 output groups
SLAB_BUFS = 2
CHUNK_BUFS = 4
OUT_BUFS = 4
PSB = 8                       # psum banks in rotation
XT_LANES = 4                  # slab DMA sem lanes (> SLAB_BUFS)
PIECE_LANES = 8               # W piece DMA sem lanes (> CHUNK_BUFS)
OD_LANES = 8                  # out DMA sem lanes (> OUT_BUFS)


def _build_program(reps=1):
    """reps > 1 repeats the whole pipeline inside one NEFF (for timing:
    T(reps=a) - T(reps=b) isolates (a-b) kernel bodies from dispatch
    overhead). Functionally identical output (each rep overwrites out)."""
    nc = bass.Bass()
    # xT arrives in slab-major layout: xT_host[s, p, ko, i] = x[s*128+i, ko*128+p]
    # so each slab DMA reads 128 partitions x 16KB fully contiguous.
    xT = nc.declare_dram_parameter("xT", [MS * P, KT * P], f32, isOutput=False)
    base = nc.declare_dram_parameter("base", [K, NC], f32, isOutput=False)
    mask = nc.declare_dram_parameter("mask", [K, NC], i32, isOutput=False)
    coeff = nc.declare_dram_parameter("coeff", [P, 1], f32, isOutput=False)
    out = nc.declare_dram_parameter("out", [M, NC], f32, isOutput=True)

    xT3 = xT.rearrange("(s p) (ko i) -> s p ko i", p=P, i=P)
    base3 = base.rearrange("(ko p) n -> p ko n", p=P)
    mask3 = mask.rearrange("(ko p) n -> p ko n", p=P)
    out3 = out.rearrange("(mo p) n -> p mo n", p=P)

    with contextlib.ExitStack() as ctx:
        s_cdma = ctx.enter_context(nc.semaphore("s_cdma"))
        s_c2 = ctx.enter_context(nc.semaphore("s_c2"))
        s_xt = [ctx.enter_context(nc.semaphore(f"s_xt{i}"))
                for i in range(XT_LANES)]
        s_b = [ctx.enter_context(nc.semaphore(f"s_b{i}"))
               for i in range(PIECE_LANES)]
        s_m = [ctx.enter_context(nc.semaphore(f"s_m{i}"))
               for i in range(PIECE_LANES)]
        s_od = [ctx.enter_context(nc.semaphore(f"s_od{i}"))
                for i in range(OD_LANES)]
        s_s = ctx.enter_context(nc.semaphore("s_s"))      # ACT s-op done (1/piece)
        s_w = ctx.enter_context(nc.semaphore("s_w"))      # DVE w-op done (1/piece)
        s_xtc = ctx.enter_context(nc.semaphore("s_xtc"))  # DVE slab cvt done (1/slab)
        s_mm = ctx.enter_context(nc.semaphore("s_mm"))    # PE group done (1/group)
        s_oc = ctx.enter_context(nc.semaphore("s_oc"))    # ACT out-copy done (1/group)

        w_sb = ctx.enter_context(nc.sbuf_tensor("w_sb", [P, KT, NC], fp16))
        xt_raw = ctx.enter_context(
            nc.sbuf_tensor("xt_raw", [P, SLAB_BUFS, KT, P], f32))
        xt_sb = ctx.enter_context(
            nc.sbuf_tensor("xt_sb", [P, SLAB_BUFS, KT, P], fp16))
        b_sb = ctx.enter_context(nc.sbuf_tensor("b_sb", [P, CHUNK_BUFS, NC], f32))
        m_sb = ctx.enter_context(nc.sbuf_tensor("m_sb", [P, CHUNK_BUFS, NC], i32))
        sa_sb = ctx.enter_context(nc.sbuf_tensor("sa_sb", [P, CHUNK_BUFS, NC], f32))
        o_sb = ctx.enter_context(nc.sbuf_tensor("o_sb", [P, OUT_BUFS, NT], f32))
        c_sb = ctx.enter_context(nc.sbuf_tensor("c_sb", [P, 1], f32))
        c2_sb = ctx.enter_context(nc.sbuf_tensor("c2_sb", [P, 1], f32))
        cn_sb = ctx.enter_context(nc.sbuf_tensor("cn_sb", [P, 1], f32))
        ps = [
            ctx.enter_context(nc.psum_tensor(f"ps{i}", [P, NT], f32))
            for i in range(PSB)
        ]

        with nc.Block() as block:

            @block.sync
            def _(sync):
                sync.dma_start(c_sb[:], coeff[:]).then_inc(s_cdma, 16)
                for it in range(reps):
                    bW = it * N_PIECES          # s_s/s_w base
                    bX = it * MS                # slab count base
                    bL = it * (MS // XT_LANES) * 16   # per-lane slab base
                    bP = it * (N_PIECES // PIECE_LANES) * 16
                    if it > 0:
                        # serialize rep boundaries so per-body timing equals a
                        # single-shot run (also keeps w_sb write/read ordered)
                        sync.wait_ge(s_oc, it * N_GROUPS)
                    # first slabs of this rep (slot free once cvt of s-2 done)
                    for s in range(min(SLAB_BUFS, MS)):
                        if bX + s >= SLAB_BUFS:
                            sync.wait_ge(s_xtc, bX + s - SLAB_BUFS + 1)
                        sync.dma_start(
                            xt_raw[:, s % SLAB_BUFS], xT3[s]
                        ).then_inc(s_xt[s % XT_LANES], 16)
                    # W pieces: one full-width [128, NC] piece per k
                    for j in range(N_PIECES):
                        if bW + j >= CHUNK_BUFS:
                            sync.wait_ge(s_w, bW + j - CHUNK_BUFS + 1)
                            sync.wait_ge(s_s, bW + j - CHUNK_BUFS + 1)
                        sync.dma_start(
                            b_sb[:, j % CHUNK_BUFS], base3[:, j],
                        ).then_inc(s_b[j % PIECE_LANES], 16)
                        sync.dma_start(
                            m_sb[:, j % CHUNK_BUFS], mask3[:, j],
                        ).then_inc(s_m[j % PIECE_LANES], 16)
                    # remaining slabs
                    for s in range(SLAB_BUFS, MS):
                        sync.wait_ge(s_xtc, bX + s - SLAB_BUFS + 1)
                        sync.dma_start(
                            xt_raw[:, s % SLAB_BUFS], xT3[s]
                        ).then_inc(s_xt[s % XT_LANES], 16)

            @block.scalar
            def _(scalar):
                scalar.wait_ge(s_cdma, 16)
                scalar.activation(c2_sb[:], c_sb[:], Copy, scale=2.0)
                scalar.activation(cn_sb[:], c_sb[:], Copy, scale=-1.0) \
                    .then_inc(s_c2, 1)
                # scale/bias operands are fetched at dispatch; wait for our own
                # writes to drain before the first use
                scalar.wait_ge(s_c2, 1)
                for it in range(reps):
                    bW = it * N_PIECES
                    bG = it * N_GROUPS
                    bP = it * (N_PIECES // PIECE_LANES) * 16
                    bO = it * (N_GROUPS // OD_LANES) * 16
                    for j in range(N_PIECES):
                        scalar.wait_ge(s_m[j % PIECE_LANES],
                                       bP + 16 * (j // PIECE_LANES + 1))
                        if bW + j >= CHUNK_BUFS:
                            scalar.wait_ge(s_w, bW + j - CHUNK_BUFS + 1)
                        scalar.activation(
                            sa_sb[:, j % CHUNK_BUFS], m_sb[:, j % CHUNK_BUFS],
                            Identity, scale=c2_sb[:], bias=cn_sb[:],
                        ).then_inc(s_s, 1)
                    # PSUM -> SBUF copies
                    for g in range(N_GROUPS):
                        scalar.wait_ge(s_mm, bG + g + 1)
                        if bG + g >= OUT_BUFS:
                            gp = bG + g - OUT_BUFS
                            scalar.wait_ge(s_od[gp % OD_LANES],
                                           16 * (gp // OD_LANES + 1))
                        scalar.copy(o_sb[:, g % OUT_BUFS], ps[g % PSB][:]) \
                            .then_inc(s_oc, 1)

            @block.vector
            def _(vector):
                for it in range(reps):
                    bW = it * N_PIECES
                    bX = it * MS
                    bL = it * (MS // XT_LANES) * 16
                    bP = it * (N_PIECES // PIECE_LANES) * 16

                    def convert_slab(s, bX=bX, bL=bL, it=it):
                        vector.wait_ge(s_xt[s % XT_LANES],
                                       bL + 16 * (s // XT_LANES + 1))
                        if bX + s >= SLAB_BUFS:
                            # fp16 slot reuse: strip s-SLAB_BUFS consumed by PE
                            vector.wait_ge(s_mm, NH * (bX + s - SLAB_BUFS + 1))
                        vector.tensor_copy(
                            xt_sb[:, s % SLAB_BUFS], xt_raw[:, s % SLAB_BUFS]
                        ).then_inc(s_xtc, 1)

                    convert_slab(0)
                    convert_slab(1)
                    for j in range(N_PIECES):
                        vector.wait_ge(s_s, bW + j + 1)
                        vector.wait_ge(s_b[j % PIECE_LANES],
                                       bP + 16 * (j // PIECE_LANES + 1))
                        vector.tensor_tensor(
                            w_sb[:, j, :],
                            sa_sb[:, j % CHUNK_BUFS], b_sb[:, j % CHUNK_BUFS],
                            mybir.AluOpType.add,
                        ).then_inc(s_w, 1)
                    for s in range(SLAB_BUFS, MS):
                        convert_slab(s)

            @block.tensor
            def _(tensor):
                for it in range(reps):
                    bW = it * N_PIECES
                    bX = it * MS
                    bG = it * N_GROUPS
                    # strip 0: k-major so matmuls chase the W build
                    # strips 0+1 fused k-major across 4 psum banks: 4 mms
                    # (0.83us) of PE work per W piece keeps PE busy while the
                    # W build streams in
                    tensor.wait_ge(s_xtc, bX + 2)
                    for k in range(KT):
                        tensor.wait_ge(s_w, bW + k + 1)
                        for st in (0, 1):
                            for h in range(NH):
                                g = bG + NH * st + h
                                if k == 0 and g >= PSB:
                                    tensor.wait_ge(s_oc, g - PSB + 1)
                                mm = tensor.matmul(
                                    ps[g % PSB][:], xt_sb[:, st, k, :],
                                    w_sb[:, k, h * NT:(h + 1) * NT],
                                    start=(k == 0), stop=(k == KT - 1),
                                )
                                if k == KT - 1:
                                    # stops fire in group order 0,1,2,3
                                    mm.then_inc(s_mm, 1)
                    for strip in range(2, MS):
                        tensor.wait_ge(s_xtc, bX + strip + 1)
                        for h in range(NH):
                            g = bG + NH * strip + h
                            if g >= PSB:
                                tensor.wait_ge(s_oc, g - PSB + 1)
                            for k in range(KT):
                                mm = tensor.matmul(
                                    ps[g % PSB][:],
                                    xt_sb[:, strip % SLAB_BUFS, k, :],
                                    w_sb[:, k, h * NT:(h + 1) * NT],
                                    start=(k == 0), stop=(k == KT - 1),
                                )
                                if k == KT - 1:
                                    mm.then_inc(s_mm, 1)

            @block.gpsimd
            def _(gpsimd):
                for it in range(reps):
                    bG = it * N_GROUPS
                    for g in range(N_GROUPS):
                        strip, h = g // NH, g % NH
                        gpsimd.wait_ge(s_oc, bG + g + 1)
                        gpsimd.dma_start(
                            out3[:, strip, h * NT:(h + 1) * NT],
                            o_sb[:, g % OUT_BUFS],
                        ).then_inc(s_od[g % OD_LANES], 16)
                for i in range(OD_LANES):
                    cnt = (reps * N_GROUPS - 1 - i) // OD_LANES + 1
                    gpsimd.wait_ge(s_od[i], 16 * cnt)

    return nc


_PROG = None


def kernel(x, base, coeff, mask):
    global _PROG
    if _PROG is None:
        _PROG = _build_program()

    x = np.asarray(x, dtype=np.float32).reshape(ROWS, K)
    base = np.asarray(base, dtype=np.float32)
    mask = np.asarray(mask, dtype=np.int32)
    coeff_np = np.full((P, 1), np.float32(coeff), dtype=np.float32)

    in_maps = []
    shard_ids = []
    for r in range(R_SHARDS):
        x_r = x[r * M:(r + 1) * M, :]
        # slab-major: [s, p, ko, i] = x_r[s*128+i, ko*128+p]
        xT_r = np.ascontiguousarray(
            x_r.reshape(MS, P, KT, P).transpose(0, 3, 2, 1)
        ).reshape(MS * P, KT * P)
        for c in range(C_SHARDS):
            in_maps.append({
                "xT": xT_r,
                "base": np.ascontiguousarray(base[:, c * NC:(c + 1) * NC]),
                "mask": np.ascontiguousarray(mask[:, c * NC:(c + 1) * NC]),
                "coeff": coeff_np,
            })
            shard_ids.append((r, c))

    res = run_bass_kernel_spmd(_PROG, in_maps, list(range(8))).results

    out = np.empty((ROWS, D_OUT), dtype=np.float32)
    for i, (r, c) in enumerate(shard_ids):
        out[r * M:(r + 1) * M, c * NC:(c + 1) * NC] = res[i]["out"]
    return out.reshape(B, S, D_OUT)
